# revision 26
# baseline (speedup 1.0000x reference)
"""LookupFFN forward on 8 Trainium2 NeuronCores.

reference:
    idx = argmin_c ||x - centroids_c||^2        (exact nearest centroid)
    out = lookup_table_fc2[idx] + fc2_bias

Equivalent formulation used here:
    idx = argmax_c (x . centroids_c - ||centroids_c||^2 / 2)

Sharding: pure data-parallel. x's 16384 tokens are split 2048 per core;
centroids / table are replicated. No collectives.

Numerics: the PE's f32r matmul rounds BOTH operands to 11 mantissa
bits (RNE; verified bit-exact by probing). A plain f32r pass flips 4
of 16384 argmaxes on this data (rel err 2.2e-2 > the 2e-2 budget).
This kernel adds the centroid-side rounding residual back with a
cheap fp8 pass:

    scores = rne11(x) . rne11(c)            f32r matmul, full rate
           + (x/256) . (c_lo*256)           e5m2 DoubleRow, 2x rate
           - |c|^2/2

where c_lo = c - rne11(c). The scale split keeps both fp8 operands in
e5m2 range, and the product lands at true scale, so the correction
accumulates into the SAME PSUM group — no extra vector work. This
leaves only the x-side rne11 noise: 1 flip on this data (rel 1.1e-2,
verified by exact host emulation that reproduces HW bit-for-bit).

Performance structure (per core: 16 token tiles of 128):
  - x staged per-tile-contiguous so every DMA descriptor is a 4KB run;
    preload issue order interleaves ct chunks with phase-A x tiles
    (DMA bandwidth is fair-shared, issue order biases completion).
  - PE p-state warm-up: dummy matmuls during the preload window so the
    first real matmul runs at full clock (~4us saved vs ramping on
    real work).
  - phase A runs the first NI tiles k-major so each arriving centroid
    chunk unlocks work on NI tiles; the Tensor stream then runs at
    ~99.6% occupancy to the end.
  - per-PSUM-bank accumulation tiles: each tile's bank-0 epilogue
    (bias-add + max) overlaps bank 1's correction matmuls.
  - bias-folded table stored fp16: gather + output stream are 4MB
    each; host upcasts the result (~3e-4 relative error, negligible).
  - engines: matmuls (Tensor), adds/maxes/argmax (DVE), row gather
    (Pool SWDGE), output copy (Scalar DGE), x feed (Sync DGE) — all
    pipelined across tiles.

Measured: 115.8us on HW (baseline hi/lo two-pass kernel: 193.8us);
Tensor stream 87.2us at 99.6% occupancy ~= the instruction floor.
"""

import numpy as np
import ml_dtypes

import bass_rust
import concourse.bass as bass
from concourse import mybir
from concourse.bass import IndirectOffsetOnAxis
from concourse.bass_utils import run_bass_kernel_spmd
from concourse.tile import TileContext

# Problem shape (fixed by the task).
B, S, D, C = 4, 4096, 1024, 1024
N_CORES = 8
N_TOK = B * S                    # 16384 tokens total
T_LOCAL = N_TOK // N_CORES       # 2048 tokens per core
P = 128                          # partitions
N_TILES = T_LOCAL // P           # 16 token tiles per core
KC = D // P                      # 8 contraction chunks (f32r pass)
KD = D // 256                    # 4 double-row chunks (fp8 pass)
NHALF = 512                      # matmul moving free dim (one PSUM bank)
CSHIFT = 8                       # fp8 scale split: x/2^8, c_lo*2^8

F32 = mybir.dt.float32
F32R = mybir.dt.float32r
F16 = mybir.dt.float16
F8 = mybir.dt.float8e5
U32 = mybir.dt.uint32


def _cap_sync_waits(nc: bass.Bass, limit: int = 1) -> None:
    """Cap every instruction at `limit` sem-waits.

    This walrus build rejects instructions carrying more than one
    sync-wait (setupSyncWait "Too many sync wait commands"), while
    Tile emits one wait per distinct producer lane. Excess waits are
    moved onto freshly inserted NoOp instructions of the same engine
    placed immediately before the instruction.
    """
    n = 0
    for func in nc.m.functions:
        for block in func.blocks:
            insts = list(block.instructions)
            out = []
            changed = False
            for inst in insts:
                si = inst.sync_info
                waits = list(si.on_wait) if si is not None and si.on_wait else []
                if len(waits) > limit:
                    for w in waits[:-limit]:
                        nop = mybir.InstNoOp(
                            name=f"I-capw-{n}",
                            engine=inst.engine,
                            ins=[],
                            outs=[],
                            sync_info=bass_rust.SyncInfo(
                                on_wait=[w], on_update=[]
                            ),
                        )
                        n += 1
                        nc.register_instruction(nop)
                        out.append(nop)
                    si.on_wait = waits[-limit:]
                    changed = True
                out.append(inst)
            if changed:
                block.instructions = out


def _build_bass() -> bass.Bass:
    nc = bass.Bass("TRN2", debug=False)

    # x shard pre-tiled on host: tile t, partition p (= d % 128),
    # chunk k (= d // 128), token tt. 4KB contiguous per partition.
    xt = nc.dram_tensor("xt", [N_TILES, P, KC, P], F32R, kind="ExternalInput").ap()
    # fp8 copy of x/256 in DoubleRow layout: d = kk*256 + r*128 + p
    xq = nc.dram_tensor("xq", [N_TILES, P, KD, 2, P], F8, kind="ExternalInput").ap()
    ct = nc.dram_tensor("ct", [D, C], F32R, kind="ExternalInput").ap()
    # fp8 centroid residual (c - rne11(c)) * 256, DoubleRow layout
    clo = nc.dram_tensor("clo", [P, KD, 2, C], F8, kind="ExternalInput").ap()
    nbias = nc.dram_tensor("nbias", [P, C], F32, kind="ExternalInput").ap()
    tab = nc.dram_tensor("tab", [C, D], F16, kind="ExternalInput").ap()
    out = nc.dram_tensor("out", [T_LOCAL, D], F16, kind="ExternalOutput").ap()

    NI = 3  # tiles interleaved k-major while the preload streams

    with TileContext(nc) as tc:
        with (
            tc.tile_pool(name="resident", bufs=1) as res_pool,
            tc.tile_pool(name="xtiles", bufs=4) as xt_pool,
            tc.tile_pool(name="psum", bufs=7, space="PSUM") as psum_pool,
            tc.tile_pool(name="warmps", bufs=1, space="PSUM") as warm_pool,
            tc.tile_pool(name="scores", bufs=4) as scores_pool,
            tc.tile_pool(name="small", bufs=N_TILES) as small_pool,
        ):
            xt_sb = {}
            xq_sb = {}

            def load_xt(t, eng=nc.sync):
                xt_sb[t] = xt_pool.tile([P, KC, P], F32R, tag="xt_t", name=f"xt{t}")
                eng.dma_start(xt_sb[t][:], xt[t])

            def load_xq(t, eng=nc.sync):
                xq_sb[t] = xt_pool.tile([P, KD, 2, P], F8, tag="xq_t", name=f"xq{t}")
                eng.dma_start(xq_sb[t][:], xq[t])

            # Preload: phase A's x tiles first, then the centroid chunks
            # in use order, then the fp8 correction operands and nbias
            # (needed later per tile). DMA bandwidth is fair-shared
            # across queued transfers, so issue order only biases
            # completion order; phase A's k-major interleaving keeps the
            # Tensor engine fed while the stream lands.
            ct_sb = [[None, None] for _ in range(KC)]

            def load_ct(k, h):
                ck = res_pool.tile([P, NHALF], F32R, tag=f"ct{k}h{h}", name=f"ct{k}h{h}")
                nc.sync.dma_start(
                    ck[:], ct[k * P : (k + 1) * P, h * NHALF : (h + 1) * NHALF]
                )
                ct_sb[k][h] = ck

            load_ct(0, 0)
            load_xt(0)
            load_ct(0, 1)
            for t in range(1, NI):
                load_xt(t)
                load_ct(t, 0)
                load_ct(t, 1)
            for k in range(NI, KC):
                load_ct(k, 0)
                load_ct(k, 1)
            clo_sb = res_pool.tile([P, KD, 2, C], F8, tag="clo")
            nc.sync.dma_start(clo_sb[:], clo[:])
            for t in range(NI):
                load_xq(t)
            load_xt(NI)
            load_xq(NI)
            nbias_sb = res_pool.tile([P, C], F32, tag="nbias")
            nc.sync.dma_start(nbias_sb[:], nbias[:])

            # PE p-state warm-up: dummy matmuls on zeroed SBUF run while
            # the preload streams, so the first real matmul starts at
            # full clock instead of paying the ~3us busy-ramp on real
            # work. They retire before ct chunk 0 lands.
            warm_sb = res_pool.tile([P, NHALF], F32R, tag="warm")
            nc.vector.memset(warm_sb[:].bitcast(F32), 0.0)
            warm_ps = warm_pool.tile([P, NHALF], F32, tag="warm_ps")
            for _ in range(16):
                nc.tensor.matmul(
                    out=warm_ps[:],
                    lhsT=warm_sb[:, 0:P],
                    rhs=warm_sb[:],
                    start=True,
                    stop=True,
                )

            def mains(t, ps, k):
                lhsT = xt_sb[t][:, k, :]
                for h in range(2):
                    nc.tensor.matmul(
                        out=ps[h][:],
                        lhsT=lhsT,
                        rhs=ct_sb[k][h][:],
                        start=(k == 0),
                        stop=False,
                    )

            def drs(t, ps):
                # centroid-residual correction: e5m2 DoubleRow into the
                # same accumulation group. Bank 0's group finishes first
                # (h-major) so the epilogue's first half overlaps bank
                # 1's correction matmuls.
                for h in range(2):
                    for kk in range(KD):
                        cols = slice(h * NHALF, (h + 1) * NHALF)
                        nc.tensor.matmul(
                            out=ps[h][:],
                            lhsT=xq_sb[t][:, kk, :, :],
                            rhs=clo_sb[:, kk, :, cols],
                            perf_mode=mybir.MatmulPerfMode.DoubleRow,
                            start=False,
                            stop=(kk == KD - 1),
                        )

            def epilogue(t, ps):
                # Per-PSUM-bank adds/maxes: the first bank's DVE work
                # hides under the second bank's correction matmuls;
                # max_index entry 0 is the global argmax.
                tok = slice(t * P, (t + 1) * P)
                sc = scores_pool.tile([P, C], F32, tag="scores_sb")
                mxh = [None, None]
                for h in range(2):
                    cols = slice(h * NHALF, (h + 1) * NHALF)
                    nc.vector.tensor_add(sc[:, cols], ps[h][:], nbias_sb[:, cols])
                    mxh[h] = small_pool.tile([P, 8], F32, tag="maxv", name=f"mxh{h}_{t}")
                    nc.vector.max(out=mxh[h][:], in_=sc[:, cols])
                mxc = small_pool.tile([P, 8], F32, tag="maxv")
                nc.vector.tensor_max(mxc[:], mxh[0][:], mxh[1][:])
                idx = small_pool.tile([P, 8], U32, tag="idx")
                nc.vector.max_index(out=idx[:], in_max=mxc[:], in_values=sc[:])

                g = scores_pool.tile([P, D], F16, tag="gath")
                nc.gpsimd.indirect_dma_start(
                    out=g[:],
                    out_offset=None,
                    in_=tab[:],
                    in_offset=IndirectOffsetOnAxis(ap=idx[:, 0:1], axis=0),
                )
                nc.scalar.dma_start(out[tok, :], g[:])
                del xt_sb[t], xq_sb[t]

            # Phase A: first NI tiles k-major, so each arriving ct chunk
            # unlocks work on NI tiles while the ladder streams.
            psA = {
                t: [psum_pool.tile([P, NHALF], F32, tag="scores_ps", name=f"psA{t}h{h}")
                    for h in range(2)]
                for t in range(NI)
            }
            for k in range(KC):
                for t in range(NI):
                    mains(t, psA[t], k)
            for t in range(NI):
                drs(t, psA[t])
                epilogue(t, psA[t])

            # Phase B: steady-state pipeline.
            for t in range(NI, N_TILES):
                if t + 1 < N_TILES:
                    load_xt(t + 1)
                    load_xq(t + 1)
                ps = [
                    psum_pool.tile([P, NHALF], F32, tag="scores_ps", name=f"ps{t}h{h}")
                    for h in range(2)
                ]
                for k in range(KC):
                    mains(t, ps, k)
                drs(t, ps)
                epilogue(t, ps)

    _cap_sync_waits(nc)
    return nc


_NC_CACHE: list = []


def _get_nc() -> bass.Bass:
    if not _NC_CACHE:
        _NC_CACHE.append(_build_bass())
    return _NC_CACHE[0]


def _rne11(a: np.ndarray) -> np.ndarray:
    """Round fp32 to 11 mantissa bits, RNE (the PE's f32r operand format)."""
    u = a.view(np.uint32).astype(np.uint64)
    half = np.uint64(1 << 11)
    lsb = (u >> np.uint64(12)) & np.uint64(1)
    u2 = (u + half - np.uint64(1) + lsb) >> np.uint64(12) << np.uint64(12)
    return u2.astype(np.uint32).view(np.float32)


def _dr_layout(a_dk: np.ndarray) -> np.ndarray:
    """[..., D] -> [..., P, KD, 2] DoubleRow operand layout, d = kk*256+r*128+p."""
    shp = a_dk.shape[:-1]
    v = a_dk.reshape(*shp, KD, 2, P)          # d = (kk, r, p)
    # want axes [..., p, kk, r]
    nd = len(shp)
    return np.ascontiguousarray(np.moveaxis(v, (nd, nd + 1, nd + 2), (nd + 1, nd + 2, nd)))


def _prepare_in_maps(x, input_centroids, lookup_table_fc2, fc2_bias):
    x = np.asarray(x, dtype=np.float32)
    cen = np.asarray(input_centroids, dtype=np.float32)
    tab = np.asarray(lookup_table_fc2, dtype=np.float32)
    bia = np.asarray(fc2_bias, dtype=np.float32)

    xf = x.reshape(N_TOK, D)

    ct = np.ascontiguousarray(cen.T)

    # centroid rounding residual, scaled into e5m2 range
    cr = _rne11(cen)
    c_lo8 = ((cen.astype(np.float64) - cr.astype(np.float64)) * (1 << CSHIFT)) \
        .astype(np.float32).astype(ml_dtypes.float8_e5m2)
    # device layout [p, kk, r, C]: value = c_lo8[c, kk*256+r*128+p]
    clo = np.ascontiguousarray(_dr_layout(c_lo8).transpose(1, 2, 3, 0))

    c_sq = np.sum(cen.astype(np.float64) ** 2, axis=1)
    nbias_row = (-0.5 * c_sq).astype(np.float32)
    nbias = np.ascontiguousarray(np.broadcast_to(nbias_row[None, :], (P, C)))

    tab16 = (tab + bia[None, :]).astype(np.float16)

    in_maps = []
    for c in range(N_CORES):
        shard = xf[c * T_LOCAL : (c + 1) * T_LOCAL]
        # [tile, p, k, tt] with d = k*128 + p, tok = tile*128 + tt
        xt = np.ascontiguousarray(
            shard.reshape(N_TILES, P, KC, P).transpose(0, 3, 2, 1)
        )
        # fp8 x/256 in DoubleRow layout [tile, p, kk, r, tt]
        x8 = (shard * (1.0 / (1 << CSHIFT))).astype(ml_dtypes.float8_e5m2)
        xq = np.ascontiguousarray(
            _dr_layout(x8.reshape(N_TILES, P, D)).transpose(0, 2, 3, 4, 1)
        )
        in_maps.append(
            {"xt": xt, "xq": xq, "ct": ct, "clo": clo,
             "nbias": nbias, "tab": tab16}
        )
    return in_maps


def run(x, input_centroids, lookup_table_fc2, fc2_bias, trace=False):
    """Run the kernel; returns (output, BassKernelResults)."""
    nc = _get_nc()
    in_maps = _prepare_in_maps(x, input_centroids, lookup_table_fc2, fc2_bias)
    res = run_bass_kernel_spmd(nc, in_maps, core_ids=list(range(N_CORES)), trace=trace)
    parts = [res.results[c]["out"] for c in range(N_CORES)]
    out = np.concatenate(parts, axis=0).astype(np.float32).reshape(B, S, D)
    return out, res


def kernel(x, input_centroids, lookup_table_fc2, fc2_bias):
    out, _ = run(x, input_centroids, lookup_table_fc2, fc2_bias, trace=False)
    return out


# revision 27
# speedup vs baseline: 1.0294x; 1.0294x over previous
"""LookupFFN forward on 8 Trainium2 NeuronCores.

reference:
    idx = argmin_c ||x - centroids_c||^2        (exact nearest centroid)
    out = lookup_table_fc2[idx] + fc2_bias

Equivalent formulation used here:
    idx = argmax_c (x . centroids_c - ||centroids_c||^2 / 2)

Sharding: pure data-parallel. x's 16384 tokens are split 2048 per core;
centroids / table are replicated. No collectives.

Numerics: the PE's f32r matmul rounds BOTH operands to 11 mantissa
bits (RNE; verified bit-exact by probing). A plain f32r pass flips 4
of 16384 argmaxes on this data (rel err 2.2e-2 > the 2e-2 budget).
This kernel adds the centroid-side rounding residual back with a
cheap fp8 pass:

    scores = rne11(x) . rne11(c)            f32r matmul, full rate
           + (x/256) . (c_lo*256)           e5m2 DoubleRow, 2x rate
           - |c|^2/2

where c_lo = c - rne11(c). The scale split keeps both fp8 operands in
e5m2 range, and the product lands at true scale, so the correction
accumulates into the SAME PSUM group — no extra vector work. This
leaves only the x-side rne11 noise: 1 flip on this data (rel 1.1e-2,
verified by exact host emulation that reproduces HW bit-for-bit).

Performance structure (per core: 16 token tiles of 128):
  - x staged per-tile-contiguous so every DMA descriptor is a 4KB run;
    preload issue order interleaves ct chunks with phase-A x tiles
    (DMA bandwidth is fair-shared, issue order biases completion).
  - PE p-state warm-up: dummy matmuls during the preload window so the
    first real matmul runs at full clock (~4us saved vs ramping on
    real work).
  - phase A runs the first NI tiles k-major so each arriving centroid
    chunk unlocks work on NI tiles; the Tensor stream then runs at
    ~99.6% occupancy to the end.
  - per-PSUM-bank accumulation tiles: each tile's bank-0 epilogue
    (bias-add + max) overlaps bank 1's correction matmuls.
  - bias-folded table stored fp16: gather + output stream are 4MB
    each; host upcasts the result (~3e-4 relative error, negligible).
  - engines: matmuls (Tensor), adds/maxes/argmax (DVE), row gather
    (Pool SWDGE), output copy (Scalar DGE), x feed (Sync DGE) — all
    pipelined across tiles.

Measured: 115.8us on HW (baseline hi/lo two-pass kernel: 193.8us);
Tensor stream 87.2us at 99.6% occupancy ~= the instruction floor.
"""

import numpy as np
import ml_dtypes

import bass_rust
import concourse.bass as bass
from concourse import mybir
from concourse.bass import IndirectOffsetOnAxis
from concourse.bass_utils import run_bass_kernel_spmd
from concourse.tile import TileContext

# Problem shape (fixed by the task).
B, S, D, C = 4, 4096, 1024, 1024
N_CORES = 8
N_TOK = B * S                    # 16384 tokens total
T_LOCAL = N_TOK // N_CORES       # 2048 tokens per core
P = 128                          # partitions
N_TILES = T_LOCAL // P           # 16 token tiles per core
KC = D // P                      # 8 contraction chunks (f32r pass)
KD = D // 256                    # 4 double-row chunks (fp8 pass)
NHALF = 512                      # matmul moving free dim (one PSUM bank)
CSHIFT = 8                       # fp8 scale split: x/2^8, c_lo*2^8

F32 = mybir.dt.float32
F32R = mybir.dt.float32r
F16 = mybir.dt.float16
F8 = mybir.dt.float8e5
U32 = mybir.dt.uint32


def _cap_sync_waits(nc: bass.Bass, limit: int = 1) -> None:
    """Cap every instruction at `limit` sem-waits.

    This walrus build rejects instructions carrying more than one
    sync-wait (setupSyncWait "Too many sync wait commands"), while
    Tile emits one wait per distinct producer lane. Excess waits are
    moved onto freshly inserted NoOp instructions of the same engine
    placed immediately before the instruction.
    """
    n = 0
    for func in nc.m.functions:
        for block in func.blocks:
            insts = list(block.instructions)
            out = []
            changed = False
            for inst in insts:
                si = inst.sync_info
                waits = list(si.on_wait) if si is not None and si.on_wait else []
                if len(waits) > limit:
                    for w in waits[:-limit]:
                        nop = mybir.InstNoOp(
                            name=f"I-capw-{n}",
                            engine=inst.engine,
                            ins=[],
                            outs=[],
                            sync_info=bass_rust.SyncInfo(
                                on_wait=[w], on_update=[]
                            ),
                        )
                        n += 1
                        nc.register_instruction(nop)
                        out.append(nop)
                    si.on_wait = waits[-limit:]
                    changed = True
                out.append(inst)
            if changed:
                block.instructions = out


def _build_bass() -> bass.Bass:
    nc = bass.Bass("TRN2", debug=False)

    # x shard pre-tiled on host: tile t, partition p (= d % 128),
    # chunk k (= d // 128), token tt. 4KB contiguous per partition.
    xt = nc.dram_tensor("xt", [N_TILES, P, KC, P], F32R, kind="ExternalInput").ap()
    # fp8 copy of x/256 in DoubleRow layout: d = kk*256 + r*128 + p
    xq = nc.dram_tensor("xq", [N_TILES, P, KD, 2, P], F8, kind="ExternalInput").ap()
    ct = nc.dram_tensor("ct", [D, C], F32R, kind="ExternalInput").ap()
    # fp8 centroid residual (c - rne11(c)) * 256, DoubleRow layout
    clo = nc.dram_tensor("clo", [P, KD, 2, C], F8, kind="ExternalInput").ap()
    nbias = nc.dram_tensor("nbias", [P, C], F32, kind="ExternalInput").ap()
    tab = nc.dram_tensor("tab", [C, D], F16, kind="ExternalInput").ap()
    out = nc.dram_tensor("out", [T_LOCAL, D], F16, kind="ExternalOutput").ap()

    NI = 3  # tiles interleaved k-major while the preload streams

    with TileContext(nc) as tc:
        with (
            tc.tile_pool(name="resident", bufs=1) as res_pool,
            tc.tile_pool(name="xtiles", bufs=4) as xt_pool,
            tc.tile_pool(name="psum", bufs=8, space="PSUM") as psum_pool,
            tc.tile_pool(name="scores", bufs=4) as scores_pool,
            tc.tile_pool(name="small", bufs=N_TILES) as small_pool,
        ):
            xt_sb = {}
            xq_sb = {}

            def load_xt(t, eng=nc.sync):
                xt_sb[t] = xt_pool.tile([P, KC, P], F32R, tag="xt_t", name=f"xt{t}")
                eng.dma_start(xt_sb[t][:], xt[t])

            def load_xq(t, eng=nc.sync):
                xq_sb[t] = xt_pool.tile([P, KD, 2, P], F8, tag="xq_t", name=f"xq{t}")
                eng.dma_start(xq_sb[t][:], xq[t])

            # Preload: phase A's x tiles first, then the centroid chunks
            # in use order, then the fp8 correction operands and nbias
            # (needed later per tile). DMA bandwidth is fair-shared
            # across queued transfers, so issue order only biases
            # completion order; phase A's k-major interleaving keeps the
            # Tensor engine fed while the stream lands.
            ct_sb = [[None, None] for _ in range(KC)]

            def load_ct(k, h):
                ck = res_pool.tile([P, NHALF], F32R, tag=f"ct{k}h{h}", name=f"ct{k}h{h}")
                nc.sync.dma_start(
                    ck[:], ct[k * P : (k + 1) * P, h * NHALF : (h + 1) * NHALF]
                )
                ct_sb[k][h] = ck

            load_ct(0, 0)
            load_xt(0)
            load_ct(0, 1)
            for t in range(1, NI):
                load_xt(t)
                load_ct(t, 0)
                load_ct(t, 1)
            for k in range(NI, KC):
                load_ct(k, 0)
                load_ct(k, 1)
            clo_sb = res_pool.tile([P, KD, 2, C], F8, tag="clo")
            nc.sync.dma_start(clo_sb[:], clo[:])
            for t in range(NI):
                load_xq(t)
            load_xt(NI)
            load_xq(NI)
            nbias_sb = res_pool.tile([P, C], F32, tag="nbias")
            nc.sync.dma_start(nbias_sb[:], nbias[:])

            # PE p-state warm-up: dummy matmuls on zeroed SBUF run while
            # the preload streams, so the first real matmul starts at
            # full clock instead of paying the ~3us busy-ramp on real
            # work. They retire before ct chunk 0 lands.
            warm_sb = res_pool.tile([P, NHALF], F32R, tag="warm")
            nc.vector.memset(warm_sb[:].bitcast(F32), 0.0)
            warm_ps = psum_pool.tile([P, NHALF], F32, tag="scores_ps", name="warm_ps")
            for _ in range(16):
                nc.tensor.matmul(
                    out=warm_ps[:],
                    lhsT=warm_sb[:, 0:P],
                    rhs=warm_sb[:],
                    start=True,
                    stop=True,
                )

            def mains(t, ps, k):
                lhsT = xt_sb[t][:, k, :]
                for h in range(2):
                    nc.tensor.matmul(
                        out=ps[h][:],
                        lhsT=lhsT,
                        rhs=ct_sb[k][h][:],
                        start=(k == 0),
                        stop=False,
                    )

            def drs(t, ps):
                # centroid-residual correction: e5m2 DoubleRow into the
                # same accumulation group. Bank 0's group finishes first
                # (h-major) so the epilogue's first half overlaps bank
                # 1's correction matmuls.
                for h in range(2):
                    for kk in range(KD):
                        cols = slice(h * NHALF, (h + 1) * NHALF)
                        nc.tensor.matmul(
                            out=ps[h][:],
                            lhsT=xq_sb[t][:, kk, :, :],
                            rhs=clo_sb[:, kk, :, cols],
                            perf_mode=mybir.MatmulPerfMode.DoubleRow,
                            start=False,
                            stop=(kk == KD - 1),
                        )

            def epilogue(t, ps):
                # Per-PSUM-bank adds/maxes: the first bank's DVE work
                # hides under the second bank's correction matmuls;
                # max_index entry 0 is the global argmax.
                tok = slice(t * P, (t + 1) * P)
                sc = scores_pool.tile([P, C], F32, tag="scores_sb")
                mxh = [None, None]
                for h in range(2):
                    cols = slice(h * NHALF, (h + 1) * NHALF)
                    nc.vector.tensor_add(sc[:, cols], ps[h][:], nbias_sb[:, cols])
                    mxh[h] = small_pool.tile([P, 8], F32, tag="maxv", name=f"mxh{h}_{t}")
                    nc.vector.max(out=mxh[h][:], in_=sc[:, cols])
                mxc = small_pool.tile([P, 8], F32, tag="maxv")
                nc.vector.tensor_max(mxc[:], mxh[0][:], mxh[1][:])
                idx = small_pool.tile([P, 8], U32, tag="idx")
                nc.vector.max_index(out=idx[:], in_max=mxc[:], in_values=sc[:])

                g = scores_pool.tile([P, D], F16, tag="gath")
                nc.gpsimd.indirect_dma_start(
                    out=g[:],
                    out_offset=None,
                    in_=tab[:],
                    in_offset=IndirectOffsetOnAxis(ap=idx[:, 0:1], axis=0),
                )
                nc.scalar.dma_start(out[tok, :], g[:])
                del xt_sb[t], xq_sb[t]

            # Phase A: first NI tiles k-major, so each arriving ct chunk
            # unlocks work on NI tiles while the ladder streams.
            psA = {
                t: [psum_pool.tile([P, NHALF], F32, tag="scores_ps", name=f"psA{t}h{h}")
                    for h in range(2)]
                for t in range(NI)
            }
            for k in range(KC):
                for t in range(NI):
                    mains(t, psA[t], k)
            for t in range(NI):
                drs(t, psA[t])
                epilogue(t, psA[t])

            # Phase B: steady-state pipeline.
            for t in range(NI, N_TILES):
                if t + 1 < N_TILES:
                    load_xt(t + 1)
                    load_xq(t + 1)
                ps = [
                    psum_pool.tile([P, NHALF], F32, tag="scores_ps", name=f"ps{t}h{h}")
                    for h in range(2)
                ]
                for k in range(KC):
                    mains(t, ps, k)
                drs(t, ps)
                epilogue(t, ps)

    _cap_sync_waits(nc)
    return nc


_NC_CACHE: list = []


def _get_nc() -> bass.Bass:
    if not _NC_CACHE:
        _NC_CACHE.append(_build_bass())
    return _NC_CACHE[0]


def _rne11(a: np.ndarray) -> np.ndarray:
    """Round fp32 to 11 mantissa bits, RNE (the PE's f32r operand format)."""
    u = a.view(np.uint32).astype(np.uint64)
    half = np.uint64(1 << 11)
    lsb = (u >> np.uint64(12)) & np.uint64(1)
    u2 = (u + half - np.uint64(1) + lsb) >> np.uint64(12) << np.uint64(12)
    return u2.astype(np.uint32).view(np.float32)


def _dr_layout(a_dk: np.ndarray) -> np.ndarray:
    """[..., D] -> [..., P, KD, 2] DoubleRow operand layout, d = kk*256+r*128+p."""
    shp = a_dk.shape[:-1]
    v = a_dk.reshape(*shp, KD, 2, P)          # d = (kk, r, p)
    # want axes [..., p, kk, r]
    nd = len(shp)
    return np.ascontiguousarray(np.moveaxis(v, (nd, nd + 1, nd + 2), (nd + 1, nd + 2, nd)))


def _prepare_in_maps(x, input_centroids, lookup_table_fc2, fc2_bias):
    x = np.asarray(x, dtype=np.float32)
    cen = np.asarray(input_centroids, dtype=np.float32)
    tab = np.asarray(lookup_table_fc2, dtype=np.float32)
    bia = np.asarray(fc2_bias, dtype=np.float32)

    xf = x.reshape(N_TOK, D)

    ct = np.ascontiguousarray(cen.T)

    # centroid rounding residual, scaled into e5m2 range
    cr = _rne11(cen)
    c_lo8 = ((cen.astype(np.float64) - cr.astype(np.float64)) * (1 << CSHIFT)) \
        .astype(np.float32).astype(ml_dtypes.float8_e5m2)
    # device layout [p, kk, r, C]: value = c_lo8[c, kk*256+r*128+p]
    clo = np.ascontiguousarray(_dr_layout(c_lo8).transpose(1, 2, 3, 0))

    c_sq = np.sum(cen.astype(np.float64) ** 2, axis=1)
    nbias_row = (-0.5 * c_sq).astype(np.float32)
    nbias = np.ascontiguousarray(np.broadcast_to(nbias_row[None, :], (P, C)))

    tab16 = (tab + bia[None, :]).astype(np.float16)

    in_maps = []
    for c in range(N_CORES):
        shard = xf[c * T_LOCAL : (c + 1) * T_LOCAL]
        # [tile, p, k, tt] with d = k*128 + p, tok = tile*128 + tt
        xt = np.ascontiguousarray(
            shard.reshape(N_TILES, P, KC, P).transpose(0, 3, 2, 1)
        )
        # fp8 x/256 in DoubleRow layout [tile, p, kk, r, tt]
        x8 = (shard * (1.0 / (1 << CSHIFT))).astype(ml_dtypes.float8_e5m2)
        xq = np.ascontiguousarray(
            _dr_layout(x8.reshape(N_TILES, P, D)).transpose(0, 2, 3, 4, 1)
        )
        in_maps.append(
            {"xt": xt, "xq": xq, "ct": ct, "clo": clo,
             "nbias": nbias, "tab": tab16}
        )
    return in_maps


def run(x, input_centroids, lookup_table_fc2, fc2_bias, trace=False):
    """Run the kernel; returns (output, BassKernelResults)."""
    nc = _get_nc()
    in_maps = _prepare_in_maps(x, input_centroids, lookup_table_fc2, fc2_bias)
    res = run_bass_kernel_spmd(nc, in_maps, core_ids=list(range(N_CORES)), trace=trace)
    parts = [res.results[c]["out"] for c in range(N_CORES)]
    out = np.concatenate(parts, axis=0).astype(np.float32).reshape(B, S, D)
    return out, res


def kernel(x, input_centroids, lookup_table_fc2, fc2_bias):
    out, _ = run(x, input_centroids, lookup_table_fc2, fc2_bias, trace=False)
    return out
